# revision 38
# baseline (speedup 1.0000x reference)
"""Phi4MM attention (qkv+LoRA, partial RoPE, GQA causal attention, o_proj+LoRA)
on 8 Trainium2 NeuronCores.

Sharding: tensor-parallel over heads. Core c owns q heads [3c, 3c+3) and kv
head c (GQA groups align), i.e. rows [3c*128,(3c+3)*128) + row blocks of k/v of
Wqkv/Bqkv, and columns [c*384,(c+1)*384) of Wo/Ao. LoRA A matrices and Bo are
replicated. Each core produces a partial o_proj output; partials are summed on
the host (the all-reduce of the TP layout).

Device layout is feature-major ("transposed"): X^T, qkv^T, attn^T with features
on SBUF partitions, so every GEMM contracts over the partition dim. o_proj is
computed token-major so the partial output DMAs out contiguously.
"""

import numpy as np
import ml_dtypes

import concourse.bacc as bacc
import concourse.mybir as mybir
import concourse.tile as tile
from concourse.bass import ts
from concourse.bass_utils import run_bass_kernel_spmd

# ---- problem constants (hardcoded per contract) ----
B, S, HIDDEN = 2, 2048, 3072
N_HEADS, N_KV, HEAD_DIM = 24, 8, 128
ROT = 96                      # rotary dims (48 pairs)
LORA_R = 256
LORA_SCALE = 2.0
ROPE_THETA = 10000.0
SCALE = HEAD_DIM ** -0.5

NC = 8                        # cores
HL = 3                        # q heads per core
T = B * S                     # 4096 tokens
TT = 512                      # token tile
NT = T // TT                  # 8 token tiles
KH = HIDDEN // 128            # 24 contraction tiles over hidden
F_CORE = HL * 128 + 2 * 128   # 640 qkv rows per core
QB = 512                      # attention q block
KB = 128                      # attention k block
NDG = QB // KB                # diagonal-straddling blocks per q row

BF16 = mybir.dt.bfloat16
F32 = mybir.dt.float32
AF = mybir.ActivationFunctionType

_cache = {}


def _build():
    nc = bacc.Bacc(None, target_bir_lowering=False)

    # ---- DRAM I/O (per-core shapes) ----
    xT = nc.dram_tensor("xT", (HIDDEN, T), BF16, kind="ExternalInput")
    wqkvT = nc.dram_tensor("wqkvT", (HIDDEN, F_CORE), BF16, kind="ExternalInput")
    aqkvT = nc.dram_tensor("aqkvT", (HIDDEN, LORA_R), BF16, kind="ExternalInput")
    bqkvT = nc.dram_tensor("bqkvT", (LORA_R, F_CORE), BF16, kind="ExternalInput")
    woT = nc.dram_tensor("woT", (HL * 128, HIDDEN), BF16, kind="ExternalInput")
    aoT = nc.dram_tensor("aoT", (HL * 128, LORA_R), BF16, kind="ExternalInput")
    boT = nc.dram_tensor("boT", (LORA_R, HIDDEN), BF16, kind="ExternalInput")
    cosF = nc.dram_tensor("cosF", (128, T), BF16, kind="ExternalInput")
    sinF = nc.dram_tensor("sinF", (128, T), BF16, kind="ExternalInput")
    rotM = nc.dram_tensor("rotM", (128, 128), BF16, kind="ExternalInput")
    consts = nc.dram_tensor("consts", (128, 2, 128), BF16, kind="ExternalInput")  # identity, ones
    masks = nc.dram_tensor("masks", (128, NDG, QB), BF16, kind="ExternalInput")
    o_part = nc.dram_tensor("o_part", (T, HIDDEN), BF16, kind="ExternalOutput")

    with tile.TileContext(nc) as tc:
        with (
            tc.tile_pool(name="persist", bufs=1) as persist,
            tc.tile_pool(name="tables", bufs=1) as tables,
        ):
            # persistent activations
            qT = persist.tile([128, HL, T], BF16, tag="qT")
            kT = persist.tile([128, T], BF16, tag="kT")
            V = persist.tile([128, T // 128, 128], BF16, tag="V")
            attnT = persist.tile([128, HL, T], BF16, tag="attnT")

            cos_sb = tables.tile([128, T], BF16, tag="cos")
            sin_sb = tables.tile([128, T], BF16, tag="sin")
            rot_sb = tables.tile([128, 128], BF16, tag="rot")
            id_sb = tables.tile([128, 128], BF16, tag="id")
            ones_sb = tables.tile([128, 128], BF16, tag="ones")
            mask_sb = tables.tile([128, NDG, QB], BF16, tag="mask")

            # ---------------- phase 1: qkv projection + rope + V transpose
            with (
                tc.tile_pool(name="wq", bufs=1) as wq,
                tc.tile_pool(name="xs", bufs=2) as xs,
                tc.tile_pool(name="sc", bufs=3) as sc,
                tc.tile_pool(name="qkv_ps", bufs=3, space="PSUM") as qkv_ps,
                tc.tile_pool(name="xa_ps", bufs=2, space="PSUM") as xa_ps,
                tc.tile_pool(name="rot_ps", bufs=2, space="PSUM") as rot_ps,
                tc.tile_pool(name="vt_ps", bufs=1, space="PSUM") as vt_ps,
            ):
                wqkv_sb = wq.tile([128, KH, F_CORE], BF16, tag="wqkv")
                aqkv_sb = wq.tile([128, KH, LORA_R], BF16, tag="aqkv")
                bqkv_sb = wq.tile([128, 2, F_CORE], BF16, tag="bqkv")
                wqr = wqkvT.rearrange("(o p) f -> p o f", p=128)
                aqr = aqkvT.rearrange("(o p) f -> p o f", p=128)
                xTr = xT.rearrange("(o p) t -> p o t", p=128)

                # issue tile-0 activations + LoRA-A weights first: the first
                # XA matmuls need only these, so PE starts ~20us earlier
                xt0 = xs.tile([128, KH, TT], BF16, tag="xt")
                for c in range(4):
                    nc.sync.dma_start(xt0[:, ts(c, KH // 4)],
                                      xTr[:, ts(c, KH // 4), ts(0, TT)])
                for c in range(2):
                    nc.sync.dma_start(aqkv_sb[:, ts(c, KH // 2)], aqr[:, ts(c, KH // 2)])
                for c in range(4):
                    nc.sync.dma_start(wqkv_sb[:, ts(c, KH // 4)], wqr[:, ts(c, KH // 4)])
                nc.sync.dma_start(bqkv_sb[:], bqkvT.rearrange("(o p) f -> p o f", p=128))
                # tables are first needed by rope/V-transpose, ~15us in
                nc.sync.dma_start(rot_sb[:], rotM[:, :])
                nc.sync.dma_start(id_sb[:], consts[:, 0])
                nc.sync.dma_start(cos_sb[:], cosF[:, :])
                nc.sync.dma_start(sin_sb[:], sinF[:, :])
                nc.sync.dma_start(ones_sb[:], consts[:, 1])
                nc.sync.dma_start(mask_sb[:], masks[:, :, :])

                for i in range(NT):
                    if i == 0:
                        xt = xt0
                    else:
                        xt = xs.tile([128, KH, TT], BF16, tag="xt")
                        for c in range(2):
                            nc.sync.dma_start(xt[:, ts(c, KH // 2)],
                                              xTr[:, ts(c, KH // 2), ts(i, TT)])

                    # LoRA A: xa^T = (X A^T)^T  [256, TT]
                    xa = sc.tile([128, 2, TT], BF16, tag="xa")
                    for rm in range(2):
                        ps = xa_ps.tile([128, TT], F32, tag="xaps")
                        for kt in range(KH):
                            nc.tensor.matmul(ps[:], aqkv_sb[:, kt, ts(rm, 128)],
                                             xt[:, kt], start=(kt == 0), stop=(kt == KH - 1))
                        nc.scalar.copy(xa[:, rm], ps[:])

                    # qkv^T m-tiles: 0..2 q heads, 3 = k, 4 = v
                    for m in range(5):
                        ps = qkv_ps.tile([128, TT], F32, tag="qkvps")
                        for kt in range(KH):
                            nc.tensor.matmul(ps[:], wqkv_sb[:, kt, ts(m, 128)],
                                             xt[:, kt], start=(kt == 0), stop=False)
                        for rm in range(2):
                            nc.tensor.matmul(ps[:], bqkv_sb[:, rm, ts(m, 128)],
                                             xa[:, rm], start=False, stop=(rm == 1))
                        s0 = sc.tile([128, TT], BF16, tag="s0")
                        nc.scalar.copy(s0[:], ps[:])
                        if m < 4:
                            # rope: out = psum*cos + (R @ s0)*sin  (R holds the
                            # rotate-half permutation incl. sign; pass rows of R
                            # are zero and of cos are one)
                            rp = rot_ps.tile([128, TT], F32, tag="rotps")
                            nc.tensor.matmul(rp[:], rot_sb[:], s0[:], start=True, stop=True)
                            r0 = sc.tile([128, TT], BF16, tag="r0")
                            nc.scalar.copy(r0[:], rp[:])
                            t1 = sc.tile([128, TT], BF16, tag="t1")
                            nc.vector.tensor_mul(t1[:], s0[:], cos_sb[:, ts(i, TT)])
                            t2 = sc.tile([128, TT], BF16, tag="t2")
                            nc.vector.tensor_mul(t2[:], r0[:], sin_sb[:, ts(i, TT)])
                            dest = qT[:, m, ts(i, TT)] if m < HL else kT[:, ts(i, TT)]
                            nc.vector.tensor_add(dest, t1[:], t2[:])
                        else:
                            # V: transpose to token-major 128-blocks
                            for b4 in range(TT // 128):
                                vp = vt_ps.tile([128, 128], BF16, tag="vtps")
                                nc.tensor.transpose(vp[:], s0[:, ts(b4, 128)], id_sb[:])
                                nc.scalar.copy(V[:, i * (TT // 128) + b4, :], vp[:])

            # ---------------- phase 2: causal GQA attention (per batch, head)
            with tc.tile_pool(name="ow", bufs=1) as ow:
                # o-proj weights: DMA overlaps with attention compute
                wo_sb = ow.tile([128, HL, HIDDEN], BF16, tag="wo")
                ao_sb = ow.tile([128, HL, LORA_R], BF16, tag="ao")
                bo_sb = ow.tile([128, 2, HIDDEN], BF16, tag="bo")
                nc.sync.dma_start(wo_sb[:], woT.rearrange("(o p) f -> p o f", p=128))
                nc.sync.dma_start(ao_sb[:], aoT.rearrange("(o p) f -> p o f", p=128))
                nc.sync.dma_start(bo_sb[:], boT.rearrange("(o p) f -> p o f", p=128))

                with (
                    tc.tile_pool(name="ps_s", bufs=4, space="PSUM") as ps_s,
                    tc.tile_pool(name="ps_at", bufs=2, space="PSUM") as ps_at,
                    tc.tile_pool(name="ps_sm", bufs=2, space="PSUM") as ps_sm,
                    tc.tile_pool(name="pp", bufs=4) as pp,
                    tc.tile_pool(name="rr", bufs=2) as rr,
                ):
                    for b in range(B):
                        for h in range(HL):
                            for qb in range(S // QB):
                                qsl = qT[:, h, b * S + qb * QB: b * S + (qb + 1) * QB]
                                at = ps_at.tile([128, QB], F32, tag="at")
                                sm = ps_sm.tile([128, QB], F32, tag="sm")
                                nkb = (qb + 1) * (QB // KB)
                                for kb in range(nkb):
                                    sp = ps_s.tile([128, QB], F32, tag="sp")
                                    nc.tensor.matmul(sp[:], kT[:, b * S + kb * KB: b * S + (kb + 1) * KB],
                                                     qsl, start=True, stop=True)
                                    pT = pp.tile([128, QB], BF16, tag="pT")
                                    nc.scalar.activation(pT[:], sp[:], AF.Exp, scale=SCALE)
                                    dg = kb - qb * (QB // KB)
                                    if dg >= 0:  # diagonal-straddling block: causal mask
                                        nc.vector.tensor_mul(pT[:], pT[:], mask_sb[:, dg, :])
                                    nc.tensor.matmul(at[:], V[:, b * (S // 128) + kb, :], pT[:],
                                                     start=(kb == 0), stop=(kb == nkb - 1))
                                    nc.tensor.matmul(sm[:], ones_sb[:], pT[:],
                                                     start=(kb == 0), stop=(kb == nkb - 1))
                                rec = rr.tile([128, QB], F32, tag="rec")
                                nc.vector.reciprocal(rec[:], sm[:])
                                nc.vector.tensor_mul(
                                    attnT[:, h, b * S + qb * QB: b * S + (qb + 1) * QB],
                                    at[:], rec[:])

                # ---------------- phase 3: o_proj (token-major) + LoRA
                with (
                    tc.tile_pool(name="xo", bufs=2) as xo,
                    tc.tile_pool(name="st", bufs=4) as stp,
                    tc.tile_pool(name="ps_xo", bufs=2, space="PSUM") as ps_xo,
                    tc.tile_pool(name="ps_o", bufs=4, space="PSUM") as ps_o,
                ):
                    FC = HIDDEN // TT  # 6 chunks of 512 output features
                    for i in range(NT):
                        xao = xo.tile([128, 2, TT], BF16, tag="xao")
                        for rm in range(2):
                            ps = ps_xo.tile([128, TT], F32, tag="xops")
                            for kt in range(HL):
                                nc.tensor.matmul(ps[:], ao_sb[:, kt, ts(rm, 128)],
                                                 attnT[:, kt, ts(i, TT)],
                                                 start=(kt == 0), stop=(kt == HL - 1))
                            nc.scalar.copy(xao[:, rm], ps[:])
                        for fc in range(FC):
                            st4 = stp.tile([128, TT // 128, TT], BF16, tag="st")
                            for tb in range(TT // 128):
                                t0 = i * TT + tb * 128
                                ps = ps_o.tile([128, TT], F32, tag="ops")
                                for kt in range(HL):
                                    nc.tensor.matmul(ps[:], attnT[:, kt, t0:t0 + 128],
                                                     wo_sb[:, kt, ts(fc, TT)],
                                                     start=(kt == 0), stop=False)
                                for rm in range(2):
                                    nc.tensor.matmul(ps[:], xao[:, rm, ts(tb, 128)],
                                                     bo_sb[:, rm, ts(fc, TT)],
                                                     start=False, stop=(rm == 1))
                                if tb % 2 == 0:
                                    nc.scalar.copy(st4[:, tb], ps[:])
                                else:
                                    nc.vector.tensor_copy(st4[:, tb], ps[:])
                            nc.sync.dma_start(
                                o_part[i * TT:(i + 1) * TT, ts(fc, TT)]
                                .rearrange("(o p) f -> p o f", p=128),
                                st4[:])

    nc.compile()
    return nc


def _host_prep(hidden_states, Wqkv, Aqkv, Bqkv, Wo, Ao, Bo, position_ids):
    bf16 = ml_dtypes.bfloat16
    X = np.asarray(hidden_states, np.float32).reshape(T, HIDDEN)
    xT = np.ascontiguousarray(X.T).astype(bf16)

    pos = np.asarray(position_ids, np.float32).reshape(T)
    inv = 1.0 / (ROPE_THETA ** (np.arange(0, ROT, 2, np.float32) / ROT))  # [48]
    ang = pos[None, :] * inv[:, None]                                     # [48, T]
    cosF = np.ones((128, T), np.float32)
    sinF = np.zeros((128, T), np.float32)
    cosF[0:48] = np.cos(ang); cosF[48:96] = np.cos(ang)
    sinF[0:48] = np.sin(ang); sinF[48:96] = np.sin(ang)
    cosF = cosF.astype(bf16)
    sinF = sinF.astype(bf16)

    rotM = np.zeros((128, 128), np.float32)  # R^T; rot = R @ q
    for f in range(48):
        rotM[48 + f, f] = -1.0               # R[f, f+48] = -1
        rotM[f, 48 + f] = 1.0                # R[f+48, f] = +1
    rotM = rotM.astype(bf16)

    consts = np.zeros((128, 2, 128), np.float32)
    consts[:, 0][np.arange(128), np.arange(128)] = 1.0  # identity
    consts[:, 1] = 1.0                                   # ones
    consts = consts.astype(bf16)

    masks = np.zeros((128, NDG, QB), np.float32)
    for dg in range(NDG):
        i = np.arange(128)[:, None]
        j = np.arange(QB)[None, :]
        masks[:, dg, :] = (dg * KB + i <= j)
    masks = masks.astype(bf16)

    aqkvT = np.ascontiguousarray(np.asarray(Aqkv, np.float32).T).astype(bf16)
    boT = np.ascontiguousarray(
        (np.asarray(Bo, np.float32) * LORA_SCALE).T).astype(bf16)

    q_pos = N_HEADS * HEAD_DIM
    kv_pos = N_KV * HEAD_DIM
    in_maps = []
    for c in range(NC):
        qr = slice(HL * 128 * c, HL * 128 * (c + 1))
        kr = slice(q_pos + 128 * c, q_pos + 128 * (c + 1))
        vr = slice(q_pos + kv_pos + 128 * c, q_pos + kv_pos + 128 * (c + 1))
        W_c = np.concatenate([Wqkv[qr], Wqkv[kr], Wqkv[vr]], 0).astype(np.float32)
        B_c = np.concatenate([Bqkv[qr], Bqkv[kr], Bqkv[vr]], 0).astype(np.float32)
        fr = slice(HL * 128 * c, HL * 128 * (c + 1))
        in_maps.append({
            "xT": xT,
            "wqkvT": np.ascontiguousarray(W_c.T).astype(bf16),
            "aqkvT": aqkvT,
            "bqkvT": np.ascontiguousarray((B_c * LORA_SCALE).T).astype(bf16),
            "woT": np.ascontiguousarray(np.asarray(Wo, np.float32)[:, fr].T).astype(bf16),
            "aoT": np.ascontiguousarray(np.asarray(Ao, np.float32)[:, fr].T).astype(bf16),
            "boT": boT,
            "cosF": cosF,
            "sinF": sinF,
            "rotM": rotM,
            "consts": consts,
            "masks": masks,
        })
    return in_maps


def kernel(hidden_states, Wqkv, Aqkv, Bqkv, Wo, Ao, Bo, position_ids):
    if "nc" not in _cache:
        _cache["nc"] = _build()
    nc = _cache["nc"]
    in_maps = _host_prep(hidden_states, Wqkv, Aqkv, Bqkv, Wo, Ao, Bo, position_ids)
    r = run_bass_kernel_spmd(nc, in_maps, core_ids=list(range(NC)))
    out = np.zeros((T, HIDDEN), np.float32)
    for c in range(NC):
        out += r.results[c]["o_part"].astype(np.float32)
    return out.reshape(B, S, HIDDEN)


# revision 39
# speedup vs baseline: 1.3365x; 1.3365x over previous
"""Phi4MM attention (qkv+LoRA, partial RoPE, GQA causal attention, o_proj+LoRA)
on 8 Trainium2 NeuronCores.

Sharding: tensor-parallel over heads. Core c owns q heads [3c, 3c+3) and kv
head c (GQA groups align), i.e. rows [3c*128,(3c+3)*128) + row blocks of k/v of
Wqkv/Bqkv, and columns [c*384,(c+1)*384) of Wo/Ao. LoRA A matrices and Bo are
replicated. Each core produces a partial o_proj output; partials are summed on
the host (the all-reduce of the TP layout).

Device layout is feature-major ("transposed"): X^T, qkv^T, attn^T with features
on SBUF partitions, so every GEMM contracts over the partition dim. o_proj is
computed token-major so the partial output DMAs out contiguously.
"""

import numpy as np
import ml_dtypes

import concourse.bacc as bacc
import concourse.mybir as mybir
import concourse.tile as tile
from concourse.bass import ts
from concourse.bass_utils import run_bass_kernel_spmd

# ---- problem constants (hardcoded per contract) ----
B, S, HIDDEN = 2, 2048, 3072
N_HEADS, N_KV, HEAD_DIM = 24, 8, 128
ROT = 96                      # rotary dims (48 pairs)
LORA_R = 256
LORA_SCALE = 2.0
ROPE_THETA = 10000.0
SCALE = HEAD_DIM ** -0.5

NC = 8                        # cores
HL = 3                        # q heads per core
T = B * S                     # 4096 tokens
TT = 512                      # token tile
NT = T // TT                  # 8 token tiles
KH = HIDDEN // 128            # 24 contraction tiles over hidden
F_CORE = HL * 128 + 2 * 128   # 640 qkv rows per core
QB = 512                      # attention q block
KB = 128                      # attention k block
NDG = QB // KB                # diagonal-straddling blocks per q row

BF16 = mybir.dt.bfloat16
F32 = mybir.dt.float32
AF = mybir.ActivationFunctionType

_cache = {}


def _build():
    nc = bacc.Bacc(None, target_bir_lowering=False)

    # ---- DRAM I/O (per-core shapes) ----
    xT = nc.dram_tensor("xT", (HIDDEN, T), BF16, kind="ExternalInput")
    wqkvT = nc.dram_tensor("wqkvT", (HIDDEN, F_CORE), BF16, kind="ExternalInput")
    aqkvT = nc.dram_tensor("aqkvT", (HIDDEN, LORA_R), BF16, kind="ExternalInput")
    bqkvT = nc.dram_tensor("bqkvT", (LORA_R, F_CORE), BF16, kind="ExternalInput")
    woT = nc.dram_tensor("woT", (HL * 128, HIDDEN), BF16, kind="ExternalInput")
    aoT = nc.dram_tensor("aoT", (HL * 128, LORA_R), BF16, kind="ExternalInput")
    boT = nc.dram_tensor("boT", (LORA_R, HIDDEN), BF16, kind="ExternalInput")
    cosF = nc.dram_tensor("cosF", (128, T), BF16, kind="ExternalInput")
    sinF = nc.dram_tensor("sinF", (128, T), BF16, kind="ExternalInput")
    rotM = nc.dram_tensor("rotM", (128, 128), BF16, kind="ExternalInput")
    consts = nc.dram_tensor("consts", (128, 2, 128), BF16, kind="ExternalInput")  # identity, ones
    masks = nc.dram_tensor("masks", (128, NDG, QB), BF16, kind="ExternalInput")
    o_part = nc.dram_tensor("o_part", (T, HIDDEN), BF16, kind="ExternalOutput")

    with tile.TileContext(nc) as tc:
        with (
            tc.tile_pool(name="persist", bufs=1) as persist,
            tc.tile_pool(name="tables", bufs=1) as tables,
        ):
            # persistent activations
            qT = persist.tile([128, HL, T], BF16, tag="qT")
            kT = persist.tile([128, T], BF16, tag="kT")
            V = persist.tile([128, T // 128, 128], BF16, tag="V")
            attnT = persist.tile([128, HL, T], BF16, tag="attnT")

            cos_sb = tables.tile([128, T], BF16, tag="cos")
            sin_sb = tables.tile([128, T], BF16, tag="sin")
            rot_sb = tables.tile([128, 128], BF16, tag="rot")
            id_sb = tables.tile([128, 128], BF16, tag="id")
            ones_sb = tables.tile([128, 128], BF16, tag="ones")
            mask_sb = tables.tile([128, NDG, QB], BF16, tag="mask")

            # ---------------- phase 1: qkv projection + rope + V transpose
            with (
                tc.tile_pool(name="wq", bufs=1) as wq,
                tc.tile_pool(name="xs", bufs=2) as xs,
                tc.tile_pool(name="sc", bufs=3) as sc,
                tc.tile_pool(name="qkv_ps", bufs=4, space="PSUM") as qkv_ps,
                tc.tile_pool(name="xa_ps", bufs=1, space="PSUM") as xa_ps,
                tc.tile_pool(name="rot_ps", bufs=2, space="PSUM") as rot_ps,
                tc.tile_pool(name="vt_ps", bufs=1, space="PSUM") as vt_ps,
            ):
                wqkv_sb = wq.tile([128, KH, F_CORE], BF16, tag="wqkv")
                aqkv_sb = wq.tile([128, KH, LORA_R], BF16, tag="aqkv")
                bqkv_sb = wq.tile([128, 2, F_CORE], BF16, tag="bqkv")
                wqr = wqkvT.rearrange("(o p) f -> p o f", p=128)
                aqr = aqkvT.rearrange("(o p) f -> p o f", p=128)
                xTr = xT.rearrange("(o p) t -> p o t", p=128)

                # issue tile-0 activations + LoRA-A weights first: the first
                # XA matmuls need only these, so PE starts ~20us earlier
                xt0 = xs.tile([128, KH, TT], BF16, tag="xt")
                for c in range(4):
                    nc.sync.dma_start(xt0[:, ts(c, KH // 4)],
                                      xTr[:, ts(c, KH // 4), ts(0, TT)])
                for c in range(2):
                    nc.sync.dma_start(aqkv_sb[:, ts(c, KH // 2)], aqr[:, ts(c, KH // 2)])
                for c in range(4):
                    nc.sync.dma_start(wqkv_sb[:, ts(c, KH // 4)], wqr[:, ts(c, KH // 4)])
                nc.sync.dma_start(bqkv_sb[:], bqkvT.rearrange("(o p) f -> p o f", p=128))
                # tables are first needed by rope/V-transpose, ~15us in
                nc.sync.dma_start(rot_sb[:], rotM[:, :])
                nc.sync.dma_start(id_sb[:], consts[:, 0])
                nc.sync.dma_start(cos_sb[:], cosF[:, :])
                nc.sync.dma_start(sin_sb[:], sinF[:, :])
                nc.sync.dma_start(ones_sb[:], consts[:, 1])
                nc.sync.dma_start(mask_sb[:], masks[:, :, :])

                for i in range(NT):
                    if i == 0:
                        xt = xt0
                    else:
                        xt = xs.tile([128, KH, TT], BF16, tag="xt")
                        for c in range(2):
                            nc.sync.dma_start(xt[:, ts(c, KH // 2)],
                                              xTr[:, ts(c, KH // 2), ts(i, TT)])

                    # LoRA A: xa^T = (X A^T)^T  [256, TT]
                    xa = sc.tile([128, 2, TT], BF16, tag="xa")
                    for rm in range(2):
                        ps = xa_ps.tile([128, TT], F32, tag="xaps")
                        for kt in range(KH):
                            nc.tensor.matmul(ps[:], aqkv_sb[:, kt, ts(rm, 128)],
                                             xt[:, kt], start=(kt == 0), stop=(kt == KH - 1))
                        nc.scalar.copy(xa[:, rm], ps[:])

                    # qkv^T m-tiles: 0..2 q heads, 3 = k, 4 = v
                    for m in range(5):
                        ps = qkv_ps.tile([128, TT], F32, tag="qkvps")
                        for kt in range(KH):
                            nc.tensor.matmul(ps[:], wqkv_sb[:, kt, ts(m, 128)],
                                             xt[:, kt], start=(kt == 0), stop=False)
                        for rm in range(2):
                            nc.tensor.matmul(ps[:], bqkv_sb[:, rm, ts(m, 128)],
                                             xa[:, rm], start=False, stop=(rm == 1))
                        s0 = sc.tile([128, TT], BF16, tag="s0")
                        nc.scalar.copy(s0[:], ps[:])
                        if m < 4:
                            # rope: out = psum*cos + (R @ s0)*sin  (R holds the
                            # rotate-half permutation incl. sign; pass rows of R
                            # are zero and of cos are one)
                            rp = rot_ps.tile([128, TT], F32, tag="rotps")
                            nc.tensor.matmul(rp[:], rot_sb[:], s0[:], start=True, stop=True)
                            r0 = sc.tile([128, TT], BF16, tag="r0")
                            nc.scalar.copy(r0[:], rp[:])
                            t1 = sc.tile([128, TT], BF16, tag="t1")
                            nc.vector.tensor_mul(t1[:], s0[:], cos_sb[:, ts(i, TT)])
                            t2 = sc.tile([128, TT], BF16, tag="t2")
                            nc.vector.tensor_mul(t2[:], r0[:], sin_sb[:, ts(i, TT)])
                            dest = qT[:, m, ts(i, TT)] if m < HL else kT[:, ts(i, TT)]
                            nc.vector.tensor_add(dest, t1[:], t2[:])
                        else:
                            # V: transpose to token-major 128-blocks
                            for b4 in range(TT // 128):
                                vp = vt_ps.tile([128, 128], BF16, tag="vtps")
                                nc.tensor.transpose(vp[:], s0[:, ts(b4, 128)], id_sb[:])
                                nc.scalar.copy(V[:, i * (TT // 128) + b4, :], vp[:])

            # ---------------- phase 2: causal GQA attention (per batch, head)
            with tc.tile_pool(name="ow", bufs=1) as ow:
                # o-proj weights: DMA overlaps with attention compute
                wo_sb = ow.tile([128, HL, HIDDEN], BF16, tag="wo")
                ao_sb = ow.tile([128, HL, LORA_R], BF16, tag="ao")
                bo_sb = ow.tile([128, 2, HIDDEN], BF16, tag="bo")
                nc.sync.dma_start(wo_sb[:], woT.rearrange("(o p) f -> p o f", p=128))
                nc.sync.dma_start(ao_sb[:], aoT.rearrange("(o p) f -> p o f", p=128))
                nc.sync.dma_start(bo_sb[:], boT.rearrange("(o p) f -> p o f", p=128))

                with (
                    tc.tile_pool(name="ps_s", bufs=4, space="PSUM") as ps_s,
                    tc.tile_pool(name="ps_at", bufs=2, space="PSUM") as ps_at,
                    tc.tile_pool(name="ps_sm", bufs=2, space="PSUM") as ps_sm,
                    tc.tile_pool(name="pp", bufs=6) as pp,
                    tc.tile_pool(name="rr", bufs=2) as rr,
                ):
                    for b in range(B):
                        for h in range(HL):
                            for qb in range(S // QB):
                                qsl = qT[:, h, b * S + qb * QB: b * S + (qb + 1) * QB]
                                at = ps_at.tile([128, QB], F32, tag="at")
                                sm = ps_sm.tile([128, QB], F32, tag="sm")
                                nkb = (qb + 1) * (QB // KB)
                                for kb in range(nkb):
                                    sp = ps_s.tile([128, QB], F32, tag="sp")
                                    nc.tensor.matmul(sp[:], kT[:, b * S + kb * KB: b * S + (kb + 1) * KB],
                                                     qsl, start=True, stop=True)
                                    pT = pp.tile([128, QB], BF16, tag="pT")
                                    nc.scalar.activation(pT[:], sp[:], AF.Exp, scale=SCALE)
                                    dg = kb - qb * (QB // KB)
                                    if dg >= 0:  # diagonal-straddling block: causal mask
                                        nc.vector.tensor_mul(pT[:], pT[:], mask_sb[:, dg, :])
                                    nc.tensor.matmul(at[:], V[:, b * (S // 128) + kb, :], pT[:],
                                                     start=(kb == 0), stop=(kb == nkb - 1))
                                    nc.tensor.matmul(sm[:], ones_sb[:], pT[:],
                                                     start=(kb == 0), stop=(kb == nkb - 1))
                                rec = rr.tile([128, QB], F32, tag="rec")
                                nc.vector.reciprocal(rec[:], sm[:])
                                nc.vector.tensor_mul(
                                    attnT[:, h, b * S + qb * QB: b * S + (qb + 1) * QB],
                                    at[:], rec[:])

                # ---------------- phase 3: o_proj (token-major) + LoRA
                with (
                    tc.tile_pool(name="xo", bufs=2) as xo,
                    tc.tile_pool(name="st", bufs=4) as stp,
                    tc.tile_pool(name="ps_xo", bufs=2, space="PSUM") as ps_xo,
                    tc.tile_pool(name="ps_o", bufs=4, space="PSUM") as ps_o,
                ):
                    FC = HIDDEN // TT  # 6 chunks of 512 output features
                    for i in range(NT):
                        xao = xo.tile([128, 2, TT], BF16, tag="xao")
                        for rm in range(2):
                            ps = ps_xo.tile([128, TT], F32, tag="xops")
                            for kt in range(HL):
                                nc.tensor.matmul(ps[:], ao_sb[:, kt, ts(rm, 128)],
                                                 attnT[:, kt, ts(i, TT)],
                                                 start=(kt == 0), stop=(kt == HL - 1))
                            nc.scalar.copy(xao[:, rm], ps[:])
                        for fc in range(FC):
                            st4 = stp.tile([128, TT // 128, TT], BF16, tag="st")
                            for tb in range(TT // 128):
                                t0 = i * TT + tb * 128
                                ps = ps_o.tile([128, TT], F32, tag="ops")
                                for kt in range(HL):
                                    nc.tensor.matmul(ps[:], attnT[:, kt, t0:t0 + 128],
                                                     wo_sb[:, kt, ts(fc, TT)],
                                                     start=(kt == 0), stop=False)
                                for rm in range(2):
                                    nc.tensor.matmul(ps[:], xao[:, rm, ts(tb, 128)],
                                                     bo_sb[:, rm, ts(fc, TT)],
                                                     start=False, stop=(rm == 1))
                                if tb % 2 == 0:
                                    nc.scalar.copy(st4[:, tb], ps[:])
                                else:
                                    nc.vector.tensor_copy(st4[:, tb], ps[:])
                            nc.sync.dma_start(
                                o_part[i * TT:(i + 1) * TT, ts(fc, TT)]
                                .rearrange("(o p) f -> p o f", p=128),
                                st4[:])

    nc.compile()
    return nc


def _host_prep(hidden_states, Wqkv, Aqkv, Bqkv, Wo, Ao, Bo, position_ids):
    bf16 = ml_dtypes.bfloat16
    X = np.asarray(hidden_states, np.float32).reshape(T, HIDDEN)
    xT = np.ascontiguousarray(X.T).astype(bf16)

    pos = np.asarray(position_ids, np.float32).reshape(T)
    inv = 1.0 / (ROPE_THETA ** (np.arange(0, ROT, 2, np.float32) / ROT))  # [48]
    ang = pos[None, :] * inv[:, None]                                     # [48, T]
    cosF = np.ones((128, T), np.float32)
    sinF = np.zeros((128, T), np.float32)
    cosF[0:48] = np.cos(ang); cosF[48:96] = np.cos(ang)
    sinF[0:48] = np.sin(ang); sinF[48:96] = np.sin(ang)
    cosF = cosF.astype(bf16)
    sinF = sinF.astype(bf16)

    rotM = np.zeros((128, 128), np.float32)  # R^T; rot = R @ q
    for f in range(48):
        rotM[48 + f, f] = -1.0               # R[f, f+48] = -1
        rotM[f, 48 + f] = 1.0                # R[f+48, f] = +1
    rotM = rotM.astype(bf16)

    consts = np.zeros((128, 2, 128), np.float32)
    consts[:, 0][np.arange(128), np.arange(128)] = 1.0  # identity
    consts[:, 1] = 1.0                                   # ones
    consts = consts.astype(bf16)

    masks = np.zeros((128, NDG, QB), np.float32)
    for dg in range(NDG):
        i = np.arange(128)[:, None]
        j = np.arange(QB)[None, :]
        masks[:, dg, :] = (dg * KB + i <= j)
    masks = masks.astype(bf16)

    aqkvT = np.ascontiguousarray(np.asarray(Aqkv, np.float32).T).astype(bf16)
    boT = np.ascontiguousarray(
        (np.asarray(Bo, np.float32) * LORA_SCALE).T).astype(bf16)

    q_pos = N_HEADS * HEAD_DIM
    kv_pos = N_KV * HEAD_DIM
    in_maps = []
    for c in range(NC):
        qr = slice(HL * 128 * c, HL * 128 * (c + 1))
        kr = slice(q_pos + 128 * c, q_pos + 128 * (c + 1))
        vr = slice(q_pos + kv_pos + 128 * c, q_pos + kv_pos + 128 * (c + 1))
        W_c = np.concatenate([Wqkv[qr], Wqkv[kr], Wqkv[vr]], 0).astype(np.float32)
        B_c = np.concatenate([Bqkv[qr], Bqkv[kr], Bqkv[vr]], 0).astype(np.float32)
        fr = slice(HL * 128 * c, HL * 128 * (c + 1))
        in_maps.append({
            "xT": xT,
            "wqkvT": np.ascontiguousarray(W_c.T).astype(bf16),
            "aqkvT": aqkvT,
            "bqkvT": np.ascontiguousarray((B_c * LORA_SCALE).T).astype(bf16),
            "woT": np.ascontiguousarray(np.asarray(Wo, np.float32)[:, fr].T).astype(bf16),
            "aoT": np.ascontiguousarray(np.asarray(Ao, np.float32)[:, fr].T).astype(bf16),
            "boT": boT,
            "cosF": cosF,
            "sinF": sinF,
            "rotM": rotM,
            "consts": consts,
            "masks": masks,
        })
    return in_maps


def kernel(hidden_states, Wqkv, Aqkv, Bqkv, Wo, Ao, Bo, position_ids):
    if "nc" not in _cache:
        _cache["nc"] = _build()
    nc = _cache["nc"]
    in_maps = _host_prep(hidden_states, Wqkv, Aqkv, Bqkv, Wo, Ao, Bo, position_ids)
    r = run_bass_kernel_spmd(nc, in_maps, core_ids=list(range(NC)))
    out = np.zeros((T, HIDDEN), np.float32)
    for c in range(NC):
        out += r.results[c]["o_part"].astype(np.float32)
    return out.reshape(B, S, HIDDEN)
